# revision 28
# baseline (speedup 1.0000x reference)
"""AttentionBlock (GroupNorm + linear attention + proj + residual) on 8 Trainium2 cores.

Reference computation (per batch element b, C=512, HW=4096):
    h   = GroupNorm32(x) * w + b
    qkv = qkv_w @ h                       (1x1 conv == channel matmul)
    q   = softmax(q, axis=spatial) * C^-0.5
    k   = softmax(k, axis=spatial)
    ctx = k @ v^T                         [C, C]
    out = proj_w @ (ctx @ q) + proj_b + x

Sharding: data-parallel over batch B=8 -> one batch element per NeuronCore.

Kernel algebra (per core):
  - softmax(q+qb) == softmax(q): per-row bias shifts cancel; only v's qkv-bias
    matters and enters as a rank-1 correction to ctx (ctx += vb[d]).
  - exp() without max-subtraction (q,k values are O(1)); softmax denominators
    (sumq, sumk) folded into row scales of small [C,C] matrices.
  - proj_w folded in early: MT = (proj_w @ ctx')^T, so the last big GEMM is
    MT @ expq and the separate proj GEMM disappears.
  - k and v are produced directly in [n, c] (transposed) layout by using the
    h-tile as the matmul's stationary operand; no explicit transposes anywhere.
  - all large matmuls run as float32r (full PE rate at N=512, ~fp32 accuracy).
"""

import os
from contextlib import ExitStack

import numpy as np

try:
    import ml_dtypes

    BF16 = np.dtype(ml_dtypes.bfloat16)
except ImportError:  # pragma: no cover
    BF16 = None

B = 8
C = 512
H = W = 64
N = H * W  # 4096 spatial positions
P = 128  # partitions
CT = C // P  # 4 channel tiles
NT = N // P  # 32 spatial tiles of 128 (for transposed k/v)
NCH = N // 512  # 8 spatial chunks of 512
GROUPS = 32
GSIZE = C // GROUPS  # 16 channels per group
EPS = 1e-5

_CACHE = {}


def _build_program():
    import concourse.bass as bass
    import concourse.tile as tile
    from concourse import bacc, mybir
    from concourse.bass import ts

    f32 = mybir.dt.float32
    f32r = mybir.dt.float32r
    bf16 = mybir.dt.bfloat16
    AF = mybir.ActivationFunctionType
    ALU = mybir.AluOpType
    AX = mybir.AxisListType

    nc = bacc.Bacc(
        "TRN2", target_bir_lowering=False, debug=False, enable_asserts=False
    )

    x_d = nc.dram_tensor("x", [C, N], f32, kind="ExternalInput").ap()
    xbf_d = nc.dram_tensor("xbf", [C, N], bf16, kind="ExternalInput").ap()
    wqkv_d = nc.dram_tensor("wqkvT", [C, 3 * C], bf16, kind="ExternalInput").ap()
    wproj_d = nc.dram_tensor("wprojT", [C, C], f32, kind="ExternalInput").ap()
    wn_d = nc.dram_tensor("wn", [CT, P], f32, kind="ExternalInput").ap()
    bn_d = nc.dram_tensor("bn", [CT, P], f32, kind="ExternalInput").ap()
    pb_d = nc.dram_tensor("pb", [CT, P], f32, kind="ExternalInput").ap()
    vbrow_d = nc.dram_tensor("vbrow", [1, C], bf16, kind="ExternalInput").ap()
    pcs_d = nc.dram_tensor("pcs", [1, C], bf16, kind="ExternalInput").ap()
    pmat_d = nc.dram_tensor("pmat", [P, P], f32, kind="ExternalInput").ap()
    ones_d = nc.dram_tensor("ones", [P, 1], f32, kind="ExternalInput").ap()
    onesb_d = nc.dram_tensor("onesb", [P, 1], bf16, kind="ExternalInput").ap()
    y_d = nc.dram_tensor("y", [C, N], f32, kind="ExternalOutput").ap()

    def r(ap):
        return ap.bitcast(f32r)

    with tile.TileContext(nc) as tc:
        with (
            tc.tile_pool(name="consts", bufs=1) as consts,
            tc.tile_pool(name="persist", bufs=1) as persist,
            ExitStack() as late_pools,
        ):
            # --- tiles for constants (DMAs for big weights emitted AFTER the
            # x loads so the x tiles win the DMA queues; weights ride gpsimd)
            wqkv_s = consts.tile([P, CT, 3 * C], bf16, name="wqkv_s")
            wproj_s = consts.tile([P, CT, C], f32, name="wproj_s")
            pmat_s = consts.tile([P, P], f32, name="pmat_s")
            vbrow_s = consts.tile([1, C], bf16, name="vbrow_s")
            pcs_s = consts.tile([1, C], bf16, name="pcs_s")
            wn_s = consts.tile([P, CT], f32, name="wn_s")
            bn_s = consts.tile([P, CT], f32, name="bn_s")
            pb_s = consts.tile([P, CT], f32, name="pb_s")
            eps_s = consts.tile([P, 1], f32, name="eps_s")
            ones_s = consts.tile([P, 1], f32, name="ones_s")
            onesb_s = consts.tile([P, 1], bf16, name="onesb_s")

            # --- long-lived tensors ---
            xr_s = persist.tile([P, CT, N], bf16, name="xr_s")  # raw x, 32KB/p
            Bb_s = persist.tile([P, CT], bf16, name="Bb_s")
            wbv_s = persist.tile([1, C], bf16, name="wbv_s")
            ctx1_s = persist.tile([P, CT, C], f32, name="ctx1_s")
            mts_s = persist.tile([P, CT, C], f32, name="mts_s")
            A_s = persist.tile([P, CT], f32, name="A_s")
            B_s = persist.tile([P, CT], f32, name="B_s")
            rk_s = persist.tile([P, CT], f32, name="rk_s")
            sumq_parts = persist.tile([P, CT, NCH], f32, name="sumq_parts")
            sumq_s = persist.tile([P, CT], f32, name="sumq_s")
            rq_s = persist.tile([P, CT], f32, name="rq_s")

            # ---------- Phase 1: GroupNorm stats; fold the affine into the
            # qkv weights (qkv = (W diag(A)) x + W B; q/k bias parts cancel in
            # their softmaxes, v's enters MT later as a rank-1 term) ----------
            with (
                tc.tile_pool(name="gn_sm", bufs=8) as gnsm,
                tc.tile_pool(name="gn_ps", bufs=2, space="PSUM") as gnps,
            ):
                # tiny consts first (pmat gates the whole GN small-op chain)
                nc.sync.dma_start(out=pmat_s, in_=pmat_d)
                nc.sync.dma_start(out=wn_s, in_=wn_d.rearrange("t p -> p t"))
                nc.sync.dma_start(out=bn_s, in_=bn_d.rearrange("t p -> p t"))
                nc.sync.dma_start(out=pb_s, in_=pb_d.rearrange("t p -> p t"))
                nc.vector.memset(eps_s, EPS)
                nc.sync.dma_start(out=r(ones_s), in_=r(ones_d))
                nc.sync.dma_start(out=onesb_s, in_=onesb_d)
                nc.sync.dma_start(out=vbrow_s, in_=vbrow_d)
                nc.sync.dma_start(out=pcs_s, in_=pcs_d)

                dma_engines = [nc.sync, nc.scalar, nc.gpsimd]
                nq = 0
                for j in range(CT):
                    for q in range(4):
                        dma_engines[nq % 3].dma_start(
                            out=xr_s[:, j, ts(q, N // 4)],
                            in_=xbf_d[ts(j, P), ts(q, N // 4)],
                        )
                        nq += 1

                # k/v weight columns right behind x (needed for phase 2a);
                # q columns + proj weights are deferred until later
                wqkv_r = wqkv_d.rearrange("(t p) o -> p t o", p=P)
                for j in range(CT):
                    dma_engines[nq % 3].dma_start(
                        out=wqkv_s[:, j, C : 3 * C], in_=wqkv_r[:, j, C : 3 * C]
                    )
                    nq += 1

                wbv_ps = gnps.tile([1, C], f32, name="wbv_ps")
                for j in range(CT):
                    bnst = gnsm.tile([P, NCH, 6], f32, name="bnst")
                    for m in range(NCH):
                        nc.vector.bn_stats(
                            out=bnst[:, m, :], in_=xr_s[:, j, ts(m, 512)]
                        )
                    mvp = gnsm.tile([P, 2], f32, name="mvp")  # mean, var
                    nc.vector.bn_aggr(out=mvp, in_=bnst)
                    stats = gnsm.tile([P, 2], f32, name="stats")  # mean, E2
                    nc.vector.tensor_copy(out=stats[:, 0:1], in_=mvp[:, 0:1])
                    nc.vector.scalar_tensor_tensor(
                        out=stats[:, 1:2],
                        in0=mvp[:, 0:1],
                        scalar=mvp[:, 0:1],
                        in1=mvp[:, 1:2],
                        op0=ALU.mult,
                        op1=ALU.add,
                    )
                    # group-sum + broadcast back to member partitions in one
                    # matmul with the block-diagonal indicator matrix
                    gps = gnps.tile([P, 2], f32, name="gps")
                    nc.tensor.matmul(
                        gps, lhsT=pmat_s, rhs=stats, start=True, stop=True
                    )
                    mv = gnsm.tile([P, 2], f32, name="mv")  # mu_g, E2_g
                    nc.vector.tensor_scalar_mul(
                        out=mv, in0=gps, scalar1=1.0 / GSIZE
                    )
                    musq = gnsm.tile([P, 1], f32, name="musq")
                    nc.vector.tensor_mul(out=musq, in0=mv[:, 0:1], in1=mv[:, 0:1])
                    var = gnsm.tile([P, 1], f32, name="var")
                    nc.vector.tensor_sub(out=var, in0=mv[:, 1:2], in1=musq)
                    std = gnsm.tile([P, 1], f32, name="std")
                    nc.scalar.activation(
                        out=std, in_=var, func=AF.Sqrt, bias=eps_s, scale=1.0
                    )
                    rstd = gnsm.tile([P, 1], f32, name="rstd")
                    nc.vector.reciprocal(out=rstd, in_=std)
                    # A = rstd*w ; B = b - mu*A
                    nc.vector.tensor_mul(
                        out=A_s[:, j : j + 1], in0=rstd, in1=wn_s[:, j : j + 1]
                    )
                    muA = gnsm.tile([P, 1], f32, name="muA")
                    nc.vector.tensor_mul(
                        out=muA, in0=mv[:, 0:1], in1=A_s[:, j : j + 1]
                    )
                    nc.vector.tensor_sub(
                        out=B_s[:, j : j + 1], in0=bn_s[:, j : j + 1], in1=muA
                    )
                    nc.vector.tensor_copy(
                        out=Bb_s[:, j : j + 1], in_=B_s[:, j : j + 1]
                    )
                    # v-bias row: (W_v B) accumulated over c-tiles (must read
                    # the unscaled weights, so emitted before the rescale)
                    nc.tensor.matmul(
                        wbv_ps,
                        lhsT=Bb_s[:, j : j + 1],
                        rhs=wqkv_s[:, j, 2 * C : 3 * C],
                        start=(j == 0),
                        stop=(j == CT - 1),
                    )
                    # fold A into the k/v weight rows (in place, bf16)
                    nc.vector.tensor_scalar_mul(
                        out=wqkv_s[:, j, C : 3 * C],
                        in0=wqkv_s[:, j, C : 3 * C],
                        scalar1=A_s[:, j : j + 1],
                    )
                nc.scalar.copy(out=wbv_s, in_=wbv_ps)

            # expq allocated only now: the stack allocator reuses the SBUF
            # freed by the phase-1 x pool (which closed above)
            eqp = late_pools.enter_context(tc.tile_pool(name="eq", bufs=1))
            expq_s = eqp.tile([P, CT, N], f32, name="expq_s")  # 64KB/p

            # deferred weight loads: q columns (rescaled on arrival), proj
            wqkv_r2 = wqkv_d.rearrange("(t p) o -> p t o", p=P)
            for j in range(CT):
                nc.gpsimd.dma_start(
                    out=wqkv_s[:, j, 0:C], in_=wqkv_r2[:, j, 0:C]
                )
                nc.vector.tensor_scalar_mul(
                    out=wqkv_s[:, j, 0:C],
                    in0=wqkv_s[:, j, 0:C],
                    scalar1=A_s[:, j : j + 1],
                )
            nc.gpsimd.dma_start(
                out=r(wproj_s), in_=r(wproj_d.rearrange("(t p) o -> p t o", p=P))
            )

            # ---------- Phase 2a: k/v (transposed) + context accumulation ----------
            with tc.tile_pool(name="ctxps", bufs=1, space="PSUM") as ctxps:
                ctx_ps = [
                    ctxps.tile([P, C], f32, name=f"ctx_ps{j}") for j in range(CT)
                ]
                sumk_ps = ctxps.tile([1, C], f32, name="sumk_ps")
                with (
                    tc.tile_pool(name="kvps", bufs=3, space="PSUM") as kvps,
                    tc.tile_pool(name="kvsb", bufs=3) as kvsb,
                ):
                    for i in range(NT):
                        kt_ps = kvps.tile([P, C], f32, name="kt_ps", tag="kv")
                        for j in range(CT):
                            nc.tensor.matmul(
                                kt_ps,
                                lhsT=xr_s[:, j, ts(i, P)],
                                rhs=wqkv_s[:, j, C : 2 * C],
                                start=(j == 0),
                                stop=(j == CT - 1),
                            )
                        ekt = kvsb.tile([P, C], bf16, name="ekt")
                        nc.scalar.activation(out=ekt, in_=kt_ps, func=AF.Exp)
                        vt_ps = kvps.tile([P, C], f32, name="vt_ps", tag="kv")
                        for j in range(CT):
                            nc.tensor.matmul(
                                vt_ps,
                                lhsT=xr_s[:, j, ts(i, P)],
                                rhs=wqkv_s[:, j, 2 * C : 3 * C],
                                start=(j == 0),
                                stop=(j == CT - 1),
                            )
                        vt = kvsb.tile([P, C], bf16, name="vt")
                        nc.vector.tensor_copy(out=vt, in_=vt_ps)
                        # row sums of expk for all 512 channels in one matmul:
                        # ones is the (1-column) stationary operand
                        nc.tensor.matmul(
                            sumk_ps,
                            lhsT=onesb_s,
                            rhs=ekt,
                            start=(i == 0),
                            stop=(i == NT - 1),
                        )
                        for j in range(CT):
                            nc.tensor.matmul(
                                ctx_ps[j],
                                lhsT=ekt[:, ts(j, P)],
                                rhs=vt,
                                start=(i == 0),
                                stop=(i == NT - 1),
                            )

                # rk = 1/sumk back in partition layout: ACT copies the psum
                # row to SBUF, PE transposes 128-slices, one wide reciprocal
                sumk_row = persist.tile([1, C], f32, name="sumk_row")
                nc.scalar.copy(out=sumk_row, in_=sumk_ps)
                with tc.tile_pool(name="tpps", bufs=1, space="PSUM") as tpps:
                    tp_ps = tpps.tile([P, CT], f32, name="tp_ps")
                    for j in range(CT):
                        nc.tensor.transpose(
                            tp_ps[:, j : j + 1],
                            sumk_row[0:1, ts(j, P)],
                            ones_s[0:1, 0:1],
                        )
                    nc.vector.reciprocal(out=rk_s, in_=tp_ps)
                for j in range(CT):
                    nc.vector.tensor_scalar_mul(
                        out=r(ctx1_s[:, j, :]),
                        in0=ctx_ps[j],
                        scalar1=rk_s[:, j : j + 1],
                    )

            # ---------- Phases 2b+3+4 (one PSUM scope: no pool barriers,
            # PE stays HAM-warm through the tail) ----------
            with (
                tc.tile_pool(name="qps", bufs=3, space="PSUM") as qps,
                tc.tile_pool(name="mtps", bufs=2, space="PSUM") as mtps,
                tc.tile_pool(name="fps", bufs=3, space="PSUM") as fps,
                tc.tile_pool(name="outp", bufs=4) as outp,
                tc.tile_pool(name="xst", bufs=2) as xst,
            ):
                for t in range(CT):
                    for m in range(NCH):
                        q_ps = qps.tile([P, 512], f32, name="q_ps")
                        for j in range(CT):
                            nc.tensor.matmul(
                                q_ps,
                                lhsT=wqkv_s[:, j, ts(t, P)],
                                rhs=xr_s[:, j, ts(m, 512)],
                                start=(j == 0),
                                stop=(j == CT - 1),
                            )
                        nc.scalar.activation(
                            out=r(expq_s[:, t, ts(m, 512)]),
                            in_=q_ps,
                            func=AF.Exp,
                            accum_out=sumq_parts[:, t, m : m + 1],
                        )
                nc.vector.tensor_reduce(
                    out=sumq_s, in_=sumq_parts, axis=AX.X, op=ALU.add
                )
                nc.vector.reciprocal(out=rq_s, in_=sumq_s)
                nc.vector.tensor_scalar_mul(
                    out=rq_s, in0=rq_s, scalar1=float(C) ** -0.5
                )

                # prefetch residual x as fp32 half-strips (overlaps phase 3)
                xstrips = []
                for s in range(2 * CT):
                    xs = xst.tile([P, N // 2], f32, name="xs")
                    eng = [nc.sync, nc.scalar, nc.gpsimd][s % 3]
                    eng.dma_start(
                        out=xs, in_=x_d[ts(s // 2, P), ts(s % 2, N // 2)]
                    )
                    xstrips.append(xs)

                # Phase 3: MT = (proj_w @ ctx')^T with row scales
                for dt in range(CT):
                    mt_ps = mtps.tile([P, C], f32, name="mt_ps")
                    for j in range(CT):
                        nc.tensor.matmul(
                            mt_ps,
                            lhsT=r(ctx1_s[:, j, ts(dt, P)]),
                            rhs=r(wproj_s[:, j, :]),
                            start=(j == 0),
                            stop=False,
                        )
                    # rank-1 v-bias terms: (qkv_b_v + W_v B)[d] * rowsum(proj)
                    nc.tensor.matmul(
                        mt_ps,
                        lhsT=vbrow_s[0:1, ts(dt, P)],
                        rhs=pcs_s,
                        start=False,
                        stop=False,
                    )
                    nc.tensor.matmul(
                        mt_ps,
                        lhsT=wbv_s[0:1, ts(dt, P)],
                        rhs=pcs_s,
                        start=False,
                        stop=True,
                    )
                    nc.vector.tensor_scalar_mul(
                        out=r(mts_s[:, dt, :]), in0=mt_ps, scalar1=rq_s[:, dt : dt + 1]
                    )

                # Phase 4: final GEMM + proj bias + residual
                for t in range(CT):
                    for m in range(NCH):
                        f_ps = fps.tile([P, 512], f32, name="f_ps")
                        for dt in range(CT):
                            nc.tensor.matmul(
                                f_ps,
                                lhsT=r(mts_s[:, dt, ts(t, P)]),
                                rhs=r(expq_s[:, dt, ts(m, 512)]),
                                start=(dt == 0),
                                stop=(dt == CT - 1),
                            )
                        ot = outp.tile([P, 512], f32, name="ot")
                        nc.vector.scalar_tensor_tensor(
                            out=ot,
                            in0=f_ps,
                            scalar=pb_s[:, t : t + 1],
                            in1=xstrips[2 * t + m // 4][:, ts(m % 4, 512)],
                            op0=ALU.add,
                            op1=ALU.add,
                        )
                        out_eng = [nc.sync, nc.scalar, nc.gpsimd][m % 3]
                        out_eng.dma_start(
                            out=y_d[ts(t, P), ts(m, 512)], in_=ot
                        )

    nc.compile()
    return nc


def kernel(x, norm_w, norm_b, qkv_w, qkv_b, proj_w, proj_b):
    from concourse.bass_utils import run_bass_kernel_spmd

    x = np.ascontiguousarray(np.asarray(x, dtype=np.float32))
    norm_w = np.asarray(norm_w, dtype=np.float32)
    norm_b = np.asarray(norm_b, dtype=np.float32)
    qkv_w = np.asarray(qkv_w, dtype=np.float32)
    qkv_b = np.asarray(qkv_b, dtype=np.float32)
    proj_w = np.asarray(proj_w, dtype=np.float32)
    proj_b = np.asarray(proj_b, dtype=np.float32)

    if "nc" not in _CACHE:
        _CACHE["nc"] = _build_program()
    nc = _CACHE["nc"]

    xf = x.reshape(B, C, N)
    wqkvT = np.ascontiguousarray(qkv_w.T).astype(BF16)  # [C, 3C] bf16
    wprojT = np.ascontiguousarray(proj_w.T)  # [C, C]
    wn = np.ascontiguousarray(norm_w.reshape(CT, P))
    bn = np.ascontiguousarray(norm_b.reshape(CT, P))
    pb = np.ascontiguousarray(proj_b.reshape(CT, P))
    vbrow = np.ascontiguousarray(qkv_b[2 * C : 3 * C].reshape(1, C)).astype(BF16)
    pcs = np.ascontiguousarray(proj_w.sum(axis=1).reshape(1, C)).astype(BF16)
    pmat = np.kron(
        np.eye(P // GSIZE, dtype=np.float32), np.ones((GSIZE, GSIZE), np.float32)
    )

    shared = {
        "wqkvT": wqkvT,
        "wprojT": wprojT,
        "wn": wn,
        "bn": bn,
        "pb": pb,
        "vbrow": vbrow,
        "pcs": pcs,
        "pmat": pmat,
        "ones": np.ones((P, 1), np.float32),
        "onesb": np.ones((P, 1), BF16),
    }
    in_maps = [
        dict(
            shared,
            x=np.ascontiguousarray(xf[b]),
            xbf=np.ascontiguousarray(xf[b]).astype(BF16),
        )
        for b in range(B)
    ]

    trace = bool(int(os.environ.get("BASS_ATTN_PROFILE", "0")))
    res = run_bass_kernel_spmd(
        nc, in_maps, core_ids=list(range(B)), trace=trace
    )
    _CACHE["last_result"] = res
    if trace and res.exec_time_ns is not None:
        print(f"HW exec time: {res.exec_time_ns} ns")

    out = np.stack([res.results[b]["y"] for b in range(B)], axis=0)
    return out.reshape(B, C, H, W)


# revision 32
# speedup vs baseline: 1.2240x; 1.2240x over previous
"""AttentionBlock (GroupNorm + linear attention + proj + residual) on 8 Trainium2 cores.

Reference computation (per batch element b, C=512, HW=4096):
    h   = GroupNorm32(x) * w + b
    qkv = qkv_w @ h                       (1x1 conv == channel matmul)
    q   = softmax(q, axis=spatial) * C^-0.5
    k   = softmax(k, axis=spatial)
    ctx = k @ v^T                         [C, C]
    out = proj_w @ (ctx @ q) + proj_b + x

Sharding: data-parallel over batch B=8 -> one batch element per NeuronCore.

Kernel algebra (per core):
  - softmax(q+qb) == softmax(q): per-row bias shifts cancel; only v's qkv-bias
    matters and enters as a rank-1 correction to ctx (ctx += vb[d]).
  - exp() without max-subtraction (q,k values are O(1)); softmax denominators
    (sumq, sumk) folded into row scales of small [C,C] matrices.
  - proj_w folded in early: MT = (proj_w @ ctx')^T, so the last big GEMM is
    MT @ expq and the separate proj GEMM disappears.
  - k and v are produced directly in [n, c] (transposed) layout by using the
    h-tile as the matmul's stationary operand; no explicit transposes anywhere.
  - all large matmuls run as float32r (full PE rate at N=512, ~fp32 accuracy).
"""

import os
from contextlib import ExitStack

import numpy as np

try:
    import ml_dtypes

    BF16 = np.dtype(ml_dtypes.bfloat16)
except ImportError:  # pragma: no cover
    BF16 = None

B = 8
C = 512
H = W = 64
N = H * W  # 4096 spatial positions
P = 128  # partitions
CT = C // P  # 4 channel tiles
NT = N // P  # 32 spatial tiles of 128 (for transposed k/v)
NCH = N // 512  # 8 spatial chunks of 512
GROUPS = 32
GSIZE = C // GROUPS  # 16 channels per group
EPS = 1e-5

_CACHE = {}


def _build_program():
    import concourse.bass as bass
    import concourse.tile as tile
    from concourse import bacc, mybir
    from concourse.bass import ts

    f32 = mybir.dt.float32
    f32r = mybir.dt.float32r
    bf16 = mybir.dt.bfloat16
    AF = mybir.ActivationFunctionType
    ALU = mybir.AluOpType
    AX = mybir.AxisListType

    nc = bacc.Bacc(
        "TRN2", target_bir_lowering=False, debug=False, enable_asserts=False
    )

    x_d = nc.dram_tensor("x", [C, N], f32, kind="ExternalInput").ap()
    xbf_d = nc.dram_tensor("xbf", [C, N], bf16, kind="ExternalInput").ap()
    wqkv_d = nc.dram_tensor("wqkvT", [C, 3 * C], bf16, kind="ExternalInput").ap()
    wproj_d = nc.dram_tensor("wprojT", [C, C], f32, kind="ExternalInput").ap()
    wn_d = nc.dram_tensor("wn", [CT, P], f32, kind="ExternalInput").ap()
    bn_d = nc.dram_tensor("bn", [CT, P], f32, kind="ExternalInput").ap()
    pb_d = nc.dram_tensor("pb", [CT, P], f32, kind="ExternalInput").ap()
    vbrow_d = nc.dram_tensor("vbrow", [1, C], bf16, kind="ExternalInput").ap()
    pcs_d = nc.dram_tensor("pcs", [1, C], bf16, kind="ExternalInput").ap()
    pmat_d = nc.dram_tensor("pmat", [P, P], f32, kind="ExternalInput").ap()
    ones_d = nc.dram_tensor("ones", [P, 1], f32, kind="ExternalInput").ap()
    onesb_d = nc.dram_tensor("onesb", [P, 1], bf16, kind="ExternalInput").ap()
    y_d = nc.dram_tensor("y", [C, N], f32, kind="ExternalOutput").ap()

    def r(ap):
        return ap.bitcast(f32r)

    with tile.TileContext(nc) as tc:
        with (
            tc.tile_pool(name="consts", bufs=1) as consts,
            tc.tile_pool(name="persist", bufs=1) as persist,
            ExitStack() as late_pools,
        ):
            # --- tiles for constants (DMAs for big weights emitted AFTER the
            # x loads so the x tiles win the DMA queues; weights ride gpsimd)
            wqkv_s = consts.tile([P, CT, 3 * C], bf16, name="wqkv_s")
            wproj_s = consts.tile([P, CT, C], f32, name="wproj_s")
            pmat_s = consts.tile([P, P], f32, name="pmat_s")
            vbrow_s = consts.tile([1, C], bf16, name="vbrow_s")
            pcs_s = consts.tile([1, C], bf16, name="pcs_s")
            wn_s = consts.tile([P, CT], f32, name="wn_s")
            bn_s = consts.tile([P, CT], f32, name="bn_s")
            pb_s = consts.tile([P, CT], f32, name="pb_s")
            eps_s = consts.tile([P, 1], f32, name="eps_s")
            ones_s = consts.tile([P, 1], f32, name="ones_s")
            onesb_s = consts.tile([P, 1], bf16, name="onesb_s")

            # --- long-lived tensors ---
            xr_s = persist.tile([P, CT, N], bf16, name="xr_s")  # raw x, 32KB/p
            xf_s = persist.tile([P, CT, N], f32, name="xf_s")  # fp32 x, 64KB/p
            Bb_s = persist.tile([P, CT], bf16, name="Bb_s")
            wbv_s = persist.tile([1, C], bf16, name="wbv_s")
            ctx1_s = persist.tile([P, CT, C], f32, name="ctx1_s")
            mts_s = persist.tile([P, CT, C], bf16, name="mts_s")
            A_s = persist.tile([P, CT], f32, name="A_s")
            B_s = persist.tile([P, CT], f32, name="B_s")
            rk_s = persist.tile([P, CT], f32, name="rk_s")
            sumq_parts = persist.tile([P, CT, NCH], f32, name="sumq_parts")
            sumq_s = persist.tile([P, CT], f32, name="sumq_s")
            rq_s = persist.tile([P, CT], f32, name="rq_s")

            # ---------- Phase 1: GroupNorm stats; fold the affine into the
            # qkv weights (qkv = (W diag(A)) x + W B; q/k bias parts cancel in
            # their softmaxes, v's enters MT later as a rank-1 term) ----------
            with (
                tc.tile_pool(name="gn_sm", bufs=8) as gnsm,
                tc.tile_pool(name="gn_ps", bufs=2, space="PSUM") as gnps,
            ):
                # tiny consts first (pmat gates the whole GN small-op chain)
                nc.sync.dma_start(out=pmat_s, in_=pmat_d)
                nc.sync.dma_start(out=wn_s, in_=wn_d.rearrange("t p -> p t"))
                nc.sync.dma_start(out=bn_s, in_=bn_d.rearrange("t p -> p t"))
                nc.sync.dma_start(out=pb_s, in_=pb_d.rearrange("t p -> p t"))
                nc.vector.memset(eps_s, EPS)
                nc.sync.dma_start(out=r(ones_s), in_=r(ones_d))
                nc.sync.dma_start(out=onesb_s, in_=onesb_d)
                nc.sync.dma_start(out=vbrow_s, in_=vbrow_d)
                nc.sync.dma_start(out=pcs_s, in_=pcs_d)

                dma_engines = [nc.sync, nc.scalar, nc.gpsimd]
                nq = 0
                for j in range(CT):
                    for q in range(4):
                        dma_engines[nq % 3].dma_start(
                            out=xr_s[:, j, ts(q, N // 4)],
                            in_=xbf_d[ts(j, P), ts(q, N // 4)],
                        )
                        nq += 1

                # k/v weight columns right behind x (needed for phase 2a);
                # q columns + proj weights are deferred until later
                wqkv_r = wqkv_d.rearrange("(t p) o -> p t o", p=P)
                for j in range(CT):
                    dma_engines[nq % 3].dma_start(
                        out=wqkv_s[:, j, C : 3 * C], in_=wqkv_r[:, j, C : 3 * C]
                    )
                    nq += 1

                wbv_ps = gnps.tile([1, C], f32, name="wbv_ps")
                for j in range(CT):
                    bnst = gnsm.tile([P, NCH, 6], f32, name="bnst")
                    for m in range(NCH):
                        nc.vector.bn_stats(
                            out=bnst[:, m, :], in_=xr_s[:, j, ts(m, 512)]
                        )
                    mvp = gnsm.tile([P, 2], f32, name="mvp")  # mean, var
                    nc.vector.bn_aggr(out=mvp, in_=bnst)
                    stats = gnsm.tile([P, 2], f32, name="stats")  # mean, E2
                    nc.vector.tensor_copy(out=stats[:, 0:1], in_=mvp[:, 0:1])
                    nc.vector.scalar_tensor_tensor(
                        out=stats[:, 1:2],
                        in0=mvp[:, 0:1],
                        scalar=mvp[:, 0:1],
                        in1=mvp[:, 1:2],
                        op0=ALU.mult,
                        op1=ALU.add,
                    )
                    # group-sum + broadcast back to member partitions in one
                    # matmul with the block-diagonal indicator matrix
                    gps = gnps.tile([P, 2], f32, name="gps")
                    nc.tensor.matmul(
                        gps, lhsT=pmat_s, rhs=stats, start=True, stop=True
                    )
                    mv = gnsm.tile([P, 2], f32, name="mv")  # mu_g, E2_g
                    nc.vector.tensor_scalar_mul(
                        out=mv, in0=gps, scalar1=1.0 / GSIZE
                    )
                    musq = gnsm.tile([P, 1], f32, name="musq")
                    nc.vector.tensor_mul(out=musq, in0=mv[:, 0:1], in1=mv[:, 0:1])
                    var = gnsm.tile([P, 1], f32, name="var")
                    nc.vector.tensor_sub(out=var, in0=mv[:, 1:2], in1=musq)
                    std = gnsm.tile([P, 1], f32, name="std")
                    nc.scalar.activation(
                        out=std, in_=var, func=AF.Sqrt, bias=eps_s, scale=1.0
                    )
                    rstd = gnsm.tile([P, 1], f32, name="rstd")
                    nc.vector.reciprocal(out=rstd, in_=std)
                    # A = rstd*w ; B = b - mu*A
                    nc.vector.tensor_mul(
                        out=A_s[:, j : j + 1], in0=rstd, in1=wn_s[:, j : j + 1]
                    )
                    muA = gnsm.tile([P, 1], f32, name="muA")
                    nc.vector.tensor_mul(
                        out=muA, in0=mv[:, 0:1], in1=A_s[:, j : j + 1]
                    )
                    nc.vector.tensor_sub(
                        out=B_s[:, j : j + 1], in0=bn_s[:, j : j + 1], in1=muA
                    )
                    nc.vector.tensor_copy(
                        out=Bb_s[:, j : j + 1], in_=B_s[:, j : j + 1]
                    )
                    # v-bias row: (W_v B) accumulated over c-tiles (must read
                    # the unscaled weights, so emitted before the rescale)
                    nc.tensor.matmul(
                        wbv_ps,
                        lhsT=Bb_s[:, j : j + 1],
                        rhs=wqkv_s[:, j, 2 * C : 3 * C],
                        start=(j == 0),
                        stop=(j == CT - 1),
                    )
                    # fold A into the k/v weight rows (in place, bf16)
                    nc.scalar.mul(
                        out=wqkv_s[:, j, C : 3 * C],
                        in_=wqkv_s[:, j, C : 3 * C],
                        mul=A_s[:, j : j + 1],
                    )
                nc.scalar.copy(out=wbv_s, in_=wbv_ps)

            # expq allocated only now: the stack allocator reuses the SBUF
            # freed by the phase-1 x pool (which closed above)
            eqp = late_pools.enter_context(tc.tile_pool(name="eq", bufs=1))
            expq_s = eqp.tile([P, CT, N], bf16, name="expq_s")  # 32KB/p

            # fp32 x for the residual: loaded during phase 2 (queues idle),
            # resident in SBUF so phase 4 needs no input DMA at all
            for s in range(2 * CT):
                eng = [nc.sync, nc.scalar, nc.gpsimd][s % 3]
                eng.dma_start(
                    out=xf_s[:, s // 2, ts(s % 2, N // 2)],
                    in_=x_d[ts(s // 2, P), ts(s % 2, N // 2)],
                )

            # deferred weight loads: q columns (rescaled on arrival), proj
            wqkv_r2 = wqkv_d.rearrange("(t p) o -> p t o", p=P)
            for j in range(CT):
                nc.gpsimd.dma_start(
                    out=wqkv_s[:, j, 0:C], in_=wqkv_r2[:, j, 0:C]
                )
                nc.scalar.mul(
                    out=wqkv_s[:, j, 0:C],
                    in_=wqkv_s[:, j, 0:C],
                    mul=A_s[:, j : j + 1],
                )
            nc.gpsimd.dma_start(
                out=r(wproj_s), in_=r(wproj_d.rearrange("(t p) o -> p t o", p=P))
            )

            # ---------- Phase 2a: k/v (transposed) + context accumulation ----------
            with tc.tile_pool(name="ctxps", bufs=1, space="PSUM") as ctxps:
                ctx_ps = [
                    ctxps.tile([P, C], f32, name=f"ctx_ps{j}") for j in range(CT)
                ]
                sumk_ps = ctxps.tile([1, C], f32, name="sumk_ps")
                with (
                    tc.tile_pool(name="kvps", bufs=3, space="PSUM") as kvps,
                    tc.tile_pool(name="kvsb", bufs=3) as kvsb,
                ):
                    for i in range(NT):
                        kt_ps = kvps.tile([P, C], f32, name="kt_ps", tag="kv")
                        for j in range(CT):
                            nc.tensor.matmul(
                                kt_ps,
                                lhsT=xr_s[:, j, ts(i, P)],
                                rhs=wqkv_s[:, j, C : 2 * C],
                                start=(j == 0),
                                stop=(j == CT - 1),
                            )
                        ekt = kvsb.tile([P, C], bf16, name="ekt")
                        nc.scalar.activation(out=ekt, in_=kt_ps, func=AF.Exp)
                        vt_ps = kvps.tile([P, C], f32, name="vt_ps", tag="kv")
                        for j in range(CT):
                            nc.tensor.matmul(
                                vt_ps,
                                lhsT=xr_s[:, j, ts(i, P)],
                                rhs=wqkv_s[:, j, 2 * C : 3 * C],
                                start=(j == 0),
                                stop=(j == CT - 1),
                            )
                        vt = kvsb.tile([P, C], bf16, name="vt")
                        nc.vector.tensor_copy(out=vt, in_=vt_ps)
                        # row sums of expk for all 512 channels in one matmul:
                        # ones is the (1-column) stationary operand
                        nc.tensor.matmul(
                            sumk_ps,
                            lhsT=onesb_s,
                            rhs=ekt,
                            start=(i == 0),
                            stop=(i == NT - 1),
                        )
                        for j in range(CT):
                            nc.tensor.matmul(
                                ctx_ps[j],
                                lhsT=ekt[:, ts(j, P)],
                                rhs=vt,
                                start=(i == 0),
                                stop=(i == NT - 1),
                            )

                # rk = 1/sumk back in partition layout: ACT copies the psum
                # row to SBUF, PE transposes 128-slices, one wide reciprocal
                sumk_row = persist.tile([1, C], f32, name="sumk_row")
                nc.scalar.copy(out=sumk_row, in_=sumk_ps)
                with tc.tile_pool(name="tpps", bufs=1, space="PSUM") as tpps:
                    tp_ps = tpps.tile([P, CT], f32, name="tp_ps")
                    for j in range(CT):
                        nc.tensor.transpose(
                            tp_ps[:, j : j + 1],
                            sumk_row[0:1, ts(j, P)],
                            ones_s[0:1, 0:1],
                        )
                    nc.vector.reciprocal(out=rk_s, in_=tp_ps)
                for j in range(CT):
                    nc.vector.tensor_scalar_mul(
                        out=r(ctx1_s[:, j, :]),
                        in0=ctx_ps[j],
                        scalar1=rk_s[:, j : j + 1],
                    )

            # ---------- Phases 2b+3+4 (one PSUM scope: no pool barriers,
            # PE stays HAM-warm through the tail) ----------
            with (
                tc.tile_pool(name="qps", bufs=3, space="PSUM") as qps,
                tc.tile_pool(name="mtps", bufs=2, space="PSUM") as mtps,
                tc.tile_pool(name="fps", bufs=3, space="PSUM") as fps,
                tc.tile_pool(name="outp", bufs=4) as outp,
            ):
                for t in range(CT):
                    for m in range(NCH):
                        q_ps = qps.tile([P, 512], f32, name="q_ps")
                        for j in range(CT):
                            nc.tensor.matmul(
                                q_ps,
                                lhsT=wqkv_s[:, j, ts(t, P)],
                                rhs=xr_s[:, j, ts(m, 512)],
                                start=(j == 0),
                                stop=(j == CT - 1),
                            )
                        nc.scalar.activation(
                            out=expq_s[:, t, ts(m, 512)],
                            in_=q_ps,
                            func=AF.Exp,
                            accum_out=sumq_parts[:, t, m : m + 1],
                        )
                nc.vector.tensor_reduce(
                    out=sumq_s, in_=sumq_parts, axis=AX.X, op=ALU.add
                )
                nc.vector.reciprocal(out=rq_s, in_=sumq_s)
                nc.vector.tensor_scalar_mul(
                    out=rq_s, in0=rq_s, scalar1=float(C) ** -0.5
                )

                # Phase 3: MT = (proj_w @ ctx')^T with row scales
                for dt in range(CT):
                    mt_ps = mtps.tile([P, C], f32, name="mt_ps")
                    for j in range(CT):
                        nc.tensor.matmul(
                            mt_ps,
                            lhsT=r(ctx1_s[:, j, ts(dt, P)]),
                            rhs=r(wproj_s[:, j, :]),
                            start=(j == 0),
                            stop=False,
                        )
                    # rank-1 v-bias terms: (qkv_b_v + W_v B)[d] * rowsum(proj)
                    nc.tensor.matmul(
                        mt_ps,
                        lhsT=vbrow_s[0:1, ts(dt, P)],
                        rhs=pcs_s,
                        start=False,
                        stop=False,
                    )
                    nc.tensor.matmul(
                        mt_ps,
                        lhsT=wbv_s[0:1, ts(dt, P)],
                        rhs=pcs_s,
                        start=False,
                        stop=True,
                    )
                    nc.vector.tensor_scalar_mul(
                        out=mts_s[:, dt, :], in0=mt_ps, scalar1=rq_s[:, dt : dt + 1]
                    )

                # Phase 4: final GEMM + proj bias + residual
                for t in range(CT):
                    for m in range(NCH):
                        f_ps = fps.tile([P, 512], f32, name="f_ps")
                        for dt in range(CT):
                            nc.tensor.matmul(
                                f_ps,
                                lhsT=mts_s[:, dt, ts(t, P)],
                                rhs=expq_s[:, dt, ts(m, 512)],
                                start=(dt == 0),
                                stop=(dt == CT - 1),
                            )
                        ot = outp.tile([P, 512], f32, name="ot")
                        nc.vector.scalar_tensor_tensor(
                            out=ot,
                            in0=f_ps,
                            scalar=pb_s[:, t : t + 1],
                            in1=xf_s[:, t, ts(m, 512)],
                            op0=ALU.add,
                            op1=ALU.add,
                        )
                        out_eng = [nc.sync, nc.scalar, nc.gpsimd][m % 3]
                        out_eng.dma_start(
                            out=y_d[ts(t, P), ts(m, 512)], in_=ot
                        )

    nc.compile()
    return nc


def kernel(x, norm_w, norm_b, qkv_w, qkv_b, proj_w, proj_b):
    from concourse.bass_utils import run_bass_kernel_spmd

    x = np.ascontiguousarray(np.asarray(x, dtype=np.float32))
    norm_w = np.asarray(norm_w, dtype=np.float32)
    norm_b = np.asarray(norm_b, dtype=np.float32)
    qkv_w = np.asarray(qkv_w, dtype=np.float32)
    qkv_b = np.asarray(qkv_b, dtype=np.float32)
    proj_w = np.asarray(proj_w, dtype=np.float32)
    proj_b = np.asarray(proj_b, dtype=np.float32)

    if "nc" not in _CACHE:
        _CACHE["nc"] = _build_program()
    nc = _CACHE["nc"]

    xf = x.reshape(B, C, N)
    wqkvT = np.ascontiguousarray(qkv_w.T).astype(BF16)  # [C, 3C] bf16
    wprojT = np.ascontiguousarray(proj_w.T)  # [C, C]
    wn = np.ascontiguousarray(norm_w.reshape(CT, P))
    bn = np.ascontiguousarray(norm_b.reshape(CT, P))
    pb = np.ascontiguousarray(proj_b.reshape(CT, P))
    vbrow = np.ascontiguousarray(qkv_b[2 * C : 3 * C].reshape(1, C)).astype(BF16)
    pcs = np.ascontiguousarray(proj_w.sum(axis=1).reshape(1, C)).astype(BF16)
    pmat = np.kron(
        np.eye(P // GSIZE, dtype=np.float32), np.ones((GSIZE, GSIZE), np.float32)
    )

    shared = {
        "wqkvT": wqkvT,
        "wprojT": wprojT,
        "wn": wn,
        "bn": bn,
        "pb": pb,
        "vbrow": vbrow,
        "pcs": pcs,
        "pmat": pmat,
        "ones": np.ones((P, 1), np.float32),
        "onesb": np.ones((P, 1), BF16),
    }
    in_maps = [
        dict(
            shared,
            x=np.ascontiguousarray(xf[b]),
            xbf=np.ascontiguousarray(xf[b]).astype(BF16),
        )
        for b in range(B)
    ]

    trace = bool(int(os.environ.get("BASS_ATTN_PROFILE", "0")))
    res = run_bass_kernel_spmd(
        nc, in_maps, core_ids=list(range(B)), trace=trace
    )
    _CACHE["last_result"] = res
    if trace and res.exec_time_ns is not None:
        print(f"HW exec time: {res.exec_time_ns} ns")

    out = np.stack([res.results[b]["y"] for b in range(B)], axis=0)
    return out.reshape(B, C, H, W)
